# revision 8
# baseline (speedup 1.0000x reference)
"""NT-Xent contrastive loss on 8 Trainium2 NeuronCores.

Math (reference): z = [z_i; z_j] (N=8192, D=128), zn = z/||z||,
sim = zn@zn.T / 0.1.  Row loss_i = logsumexp_{j!=i} sim[i,j] - sim[i, pos(i)],
loss = mean_i loss_i.

Sharding: rolled-column trick.  Core c receives z rolled by -1024*c rows.
Its 1024 local rows are rolled rows 0..1023; in rolled coordinates the
self column of local row i is i and the positive column is i + 4096 on
EVERY core, so a single static SPMD program works with no collectives.
The self logit is suppressed by adding -5 to the diagonal cosine
(logit -40 -> exp ~4e-18, negligible).  Host sums the 8 partial means.

Per-core program:
  stage A: load z (8192x128), row-normalize (sum-sq via DVE square +
           reduce_sum -- tensor_tensor_reduce hangs the DVE on HW;
           1/norm = exp(-0.5*ln(ssq)) on ACT -- same table as Exp/Ln),
           transpose via PE into znT [128(d) x 8192(rows)] in SBUF.
  main:    for each of 8 row blocks (128 rows): 16 matmuls (f32r, 512-wide)
           into [128,2048] PSUM chunks; diag shift (chunk 0) and positive
           extraction (chunk 2, DVE mul-by-identity + reduce_sum); in-place
           exp(10*cos) on ACT with per-chunk row-sum accumulation.
  epilog:  lse = ln(sum exp), row loss = lse - 10*pos_cos, reduce over
           free dim then partitions (ones-vector matmul), scale by 1/8192.
"""

import os
import sys

import numpy as np

_TRN_REPO = "/opt/trn_rl_repo"
if _TRN_REPO not in sys.path:
    sys.path.insert(0, _TRN_REPO)

from concourse import bacc, bass, mybir, tile
from concourse.bass_utils import run_bass_kernel_spmd

B = 4096
D = 128
N = 2 * B
N_CORES = 8
RPC = N // N_CORES  # 1024 rows per core
INV_T = 10.0
DIAG_SHIFT = -5.0

NBATCH = 4  # stage-A batches of 2048 rows
TPB = 16    # 128-row tiles per batch
RB = 8      # row blocks per core (128 rows each)
QB = 4      # 2048-wide column chunks
KB = 4      # 512-wide matmuls per chunk

_cache: dict = {}


def build():
    f32 = mybir.dt.float32
    f32r = mybir.dt.float32r
    AX = mybir.AxisListType
    OP = mybir.AluOpType
    AF = mybir.ActivationFunctionType

    nc = bacc.Bacc(
        "TRN2", target_bir_lowering=False, debug=False, num_devices=N_CORES
    )

    z_dram = nc.dram_tensor("z_roll", [N, D], f32, kind="ExternalInput")
    loss_dram = nc.dram_tensor("loss_part", [1, 1], f32, kind="ExternalOutput")

    eye_np = np.eye(128, dtype=np.float32)
    eye_dram = nc.inline_tensor(eye_np, name="eye128")
    negI_dram = nc.inline_tensor(
        (DIAG_SHIFT * eye_np).astype(np.float32), name="negI128"
    )
    ones_dram = nc.inline_tensor(np.ones((128, 1), np.float32), name="ones128")

    with tile.TileContext(nc) as tc:
        with (
            tc.tile_pool(name="const", bufs=1) as cpool,
            tc.tile_pool(name="zin", bufs=NBATCH) as zpool,
            tc.tile_pool(name="zn", bufs=2) as npool,
            tc.tile_pool(name="persist", bufs=1) as ppool,
            tc.tile_pool(name="scr", bufs=2) as spool,
            tc.tile_pool(name="psum", bufs=2, space=bass.MemorySpace.PSUM) as qpool,
        ):
            eye_sb = cpool.tile([128, 128], f32)
            negI_sb = cpool.tile([128, 128], f32)
            ones_sb = cpool.tile([128, 1], f32)
            nc.gpsimd.dma_start(eye_sb[:], eye_dram[:])
            nc.gpsimd.dma_start(negI_sb[:], negI_dram[:])
            nc.gpsimd.dma_start(ones_sb[:], ones_dram[:])

            # --- stage A1: load + sum of squares per row ---
            ssq = ppool.tile([128, NBATCH * TPB], f32)
            zin_tiles = []
            for b in range(NBATCH):
                zin = zpool.tile([128, TPB, 128], f32)
                zin_tiles.append(zin)
                src = z_dram[2048 * b : 2048 * (b + 1), :].rearrange(
                    "(t p) d -> p t d", p=128
                )
                nc.gpsimd.dma_start(zin[:], src)
                for t in range(TPB):
                    scr = spool.tile([128, 128], f32)
                    j = TPB * b + t
                    nc.vector.tensor_mul(scr[:], zin[:, t, :], zin[:, t, :])
                    nc.vector.reduce_sum(ssq[:, j : j + 1], scr[:], axis=AX.X)

            # 1/||z|| = exp(-0.5 * ln(ssq)); stays in the Ln/Exp ACT table.
            lnssq = ppool.tile([128, NBATCH * TPB], f32)
            inv = ppool.tile([128, NBATCH * TPB], f32)
            nc.scalar.activation(lnssq[:], ssq[:], AF.Ln)
            nc.scalar.activation(inv[:], lnssq[:], AF.Exp, scale=-0.5)

            # --- stage A2: scale rows + PE transpose into znT ---
            # znT is typed f32r: walrus requires the producer of an
            # fp32r-matmul operand to round to fp32r at write time.
            znT = ppool.tile([128, N], f32r)
            for b in range(NBATCH):
                zn = npool.tile([128, TPB, 128], f32)
                for t in range(TPB):
                    j = TPB * b + t
                    nc.vector.tensor_scalar_mul(
                        zn[:, t, :], zin_tiles[b][:, t, :], inv[:, j : j + 1]
                    )
                ps = qpool.tile([128, 2048], f32, tag="mm")
                for t in range(TPB):
                    nc.tensor.transpose(
                        ps[:, 128 * t : 128 * (t + 1)], zn[:, t, :], eye_sb[:]
                    )
                nc.vector.tensor_copy(znT[:, 2048 * b : 2048 * (b + 1)], ps[:])

            # --- main loop ---
            sexp = ppool.tile([128, RB, QB], f32)
            pos = ppool.tile([128, RB], f32)
            for r in range(RB):
                lhsT = znT[:, 128 * r : 128 * (r + 1)]
                for q in range(QB):
                    ps = qpool.tile([128, 2048], f32, tag="mm")
                    for k in range(KB):
                        c0 = 2048 * q + 512 * k
                        nc.tensor.matmul(
                            ps[:, 512 * k : 512 * (k + 1)],
                            lhsT,
                            znT[:, c0 : c0 + 512],
                            start=True,
                            stop=True,
                        )
                    if q == 0:
                        sub = ps[:, 128 * r : 128 * (r + 1)]
                        nc.vector.tensor_add(sub, sub, negI_sb[:])
                    if q == 2:
                        scr = spool.tile([128, 128], f32)
                        nc.vector.tensor_mul(
                            scr[:], ps[:, 128 * r : 128 * (r + 1)], eye_sb[:]
                        )
                        nc.vector.reduce_sum(
                            pos[:, r : r + 1], scr[:], axis=AX.X
                        )
                    nc.scalar.activation(
                        ps[:],
                        ps[:],
                        AF.Exp,
                        scale=INV_T,
                        accum_out=sexp[:, r, q : q + 1],
                    )

            # --- epilogue ---
            s8 = ppool.tile([128, RB], f32)
            nc.vector.reduce_sum(s8[:], sexp[:], axis=AX.X)
            lse = ppool.tile([128, RB], f32)
            nc.scalar.activation(lse[:], s8[:], AF.Ln)
            poss = ppool.tile([128, RB], f32)
            nc.scalar.mul(poss[:], pos[:], INV_T)
            acc = ppool.tile([128, RB], f32)
            nc.vector.tensor_sub(acc[:], lse[:], poss[:])
            tot = ppool.tile([128, 1], f32)
            nc.vector.reduce_sum(tot[:], acc[:], axis=AX.X)
            psf = qpool.tile([128, 2048], f32, tag="mm")
            nc.tensor.matmul(
                psf[0:1, 0:1], ones_sb[:], tot[:], start=True, stop=True
            )
            res = ppool.tile([1, 1], f32)
            nc.scalar.mul(res[:], psf[0:1, 0:1], 1.0 / N)
            nc.gpsimd.dma_start(loss_dram[:], res[:])

    nc.compile()
    return nc


def get_nc():
    if "nc" not in _cache:
        _cache["nc"] = build()
    return _cache["nc"]


def make_in_maps(z_i: np.ndarray, z_j: np.ndarray):
    z = np.concatenate(
        [np.asarray(z_i, np.float32), np.asarray(z_j, np.float32)], axis=0
    )
    return [
        {"z_roll": np.ascontiguousarray(np.roll(z, -RPC * c, axis=0))}
        for c in range(N_CORES)
    ]


def kernel(**inputs) -> np.ndarray:
    in_maps = make_in_maps(inputs["z_i"], inputs["z_j"])
    nc = get_nc()
    res = run_bass_kernel_spmd(nc, in_maps, list(range(N_CORES)))
    kernel.last_results = res
    total = np.float32(0.0)
    for r in res.results:
        total = np.float32(total + np.float32(np.asarray(r["loss_part"]).reshape(())))
    return np.float32(total)


# revision 10
# speedup vs baseline: 1.1404x; 1.1404x over previous
"""NT-Xent contrastive loss on 8 Trainium2 NeuronCores.

Math (reference): z = [z_i; z_j] (N=8192, D=128), zn = z/||z||,
sim = zn@zn.T / 0.1.  Row loss_i = logsumexp_{j!=i} sim[i,j] - sim[i, pos(i)],
loss = mean_i loss_i.

Sharding: rolled-column trick.  Core c receives z rolled by -1024*c rows.
Its 1024 local rows are rolled rows 0..1023; in rolled coordinates the
self column of local row i is i and the positive column is i + 4096 on
EVERY core, so a single static SPMD program works with no collectives.
The self logit is suppressed by adding -5 to the diagonal cosine
(logit -40 -> exp ~4e-18, negligible).  Host sums the 8 partial means.

Per-core schedule (V2, software-pipelined):
  prologue: batch 0 (rows 0..2047) only: DMA (4 sub-DMAs), row sum-sq on
            DVE (tensor_mul+reduce_sum -- tensor_tensor_reduce hangs the
            DVE on HW), 1/norm = exp(-0.5*ln(ssq)) on ACT, scale rows,
            PE-transpose into znT[:, :2048] (f32r producer rule).
  main:     q-outer / r-inner.  Chunk q's 8 row-block exps (ACT-bound,
            ~17us) hide batch q+1's stage-A work, interleaved into the
            engine queues mid-chunk: squares at r=0..2, norms+scale at
            r=3..4, transpose+copy halves at r=5..6.  Diag shift at q==0,
            positive extraction at q==2 (cols 4096..5119).
  epilog:   lse = ln(sum exp), row loss = lse - 10*pos_cos, reduce over
            free dim then partitions (ones-vector matmul), scale by 1/8192.
"""

import os
import sys

import numpy as np

_TRN_REPO = "/opt/trn_rl_repo"
if _TRN_REPO not in sys.path:
    sys.path.insert(0, _TRN_REPO)

from concourse import bacc, bass, mybir, tile
from concourse.bass_utils import run_bass_kernel_spmd

B = 4096
D = 128
N = 2 * B
N_CORES = 8
RPC = N // N_CORES  # 1024 rows per core
INV_T = 10.0
DIAG_SHIFT = -5.0

NBATCH = 4  # stage-A batches of 2048 rows
TPB = 16    # 128-row tiles per batch
RB = 8      # row blocks per core (128 rows each)
QB = 4      # 2048-wide column chunks
KB = 4      # 512-wide matmuls per chunk

_cache: dict = {}


def build():
    f32 = mybir.dt.float32
    f32r = mybir.dt.float32r
    AX = mybir.AxisListType
    OP = mybir.AluOpType
    AF = mybir.ActivationFunctionType

    nc = bacc.Bacc(
        "TRN2", target_bir_lowering=False, debug=False, num_devices=N_CORES
    )

    z_dram = nc.dram_tensor("z_roll", [N, D], f32, kind="ExternalInput")
    loss_dram = nc.dram_tensor("loss_part", [1, 1], f32, kind="ExternalOutput")

    eye_np = np.eye(128, dtype=np.float32)
    eye_dram = nc.inline_tensor(eye_np, name="eye128")
    negI_dram = nc.inline_tensor(
        (DIAG_SHIFT * eye_np).astype(np.float32), name="negI128"
    )
    ones_dram = nc.inline_tensor(np.ones((128, 1), np.float32), name="ones128")

    with tile.TileContext(nc) as tc:
        with (
            tc.tile_pool(name="const", bufs=1) as cpool,
            tc.tile_pool(name="zin", bufs=NBATCH) as zpool,
            tc.tile_pool(name="zn", bufs=2) as npool,
            tc.tile_pool(name="persist", bufs=1) as ppool,
            tc.tile_pool(name="scr", bufs=2) as spool,
            tc.tile_pool(name="psum", bufs=2, space=bass.MemorySpace.PSUM) as qpool,
        ):
            eye_sb = cpool.tile([128, 128], f32)
            negI_sb = cpool.tile([128, 128], f32)
            ones_sb = cpool.tile([128, 1], f32)
            nc.gpsimd.dma_start(eye_sb[:], eye_dram[:])
            nc.gpsimd.dma_start(negI_sb[:], negI_dram[:])
            nc.gpsimd.dma_start(ones_sb[:], ones_dram[:])

            ssq = ppool.tile([128, NBATCH * TPB], f32)
            lnssq = ppool.tile([128, NBATCH * TPB], f32)
            inv = ppool.tile([128, NBATCH * TPB], f32)
            # znT is typed f32r: walrus requires the producer of an
            # fp32r-matmul operand to round to fp32r at write time.
            znT = ppool.tile([128, N], f32r)
            sexp = ppool.tile([128, RB, QB], f32)
            pos = ppool.tile([128, RB], f32)

            # all input DMAs up front; batch 0 first, 4 sub-DMAs per batch
            zin_tiles = []
            for b in range(NBATCH):
                zin = zpool.tile([128, TPB, 128], f32)
                zin_tiles.append(zin)
                for s in range(4):
                    r0 = 2048 * b + 512 * s
                    src = z_dram[r0 : r0 + 512, :].rearrange(
                        "(t p) d -> p t d", p=128
                    )
                    nc.gpsimd.dma_start(zin[:, 4 * s : 4 * s + 4, :], src)

            def squares(b, t0, t1):
                for t in range(t0, t1):
                    j = TPB * b + t
                    scr = spool.tile([128, 128], f32)
                    nc.vector.tensor_mul(
                        scr[:], zin_tiles[b][:, t, :], zin_tiles[b][:, t, :]
                    )
                    nc.vector.reduce_sum(ssq[:, j : j + 1], scr[:], axis=AX.X)

            def norms(b):
                # 1/||z|| = exp(-0.5*ln(ssq)); stays in the Ln/Exp ACT table.
                j0 = TPB * b
                nc.scalar.activation(
                    lnssq[:, j0 : j0 + TPB], ssq[:, j0 : j0 + TPB], AF.Ln
                )
                nc.scalar.activation(
                    inv[:, j0 : j0 + TPB], lnssq[:, j0 : j0 + TPB],
                    AF.Exp, scale=-0.5,
                )

            zn_tiles = {}

            def tsm(b, t0, t1):
                if b not in zn_tiles:
                    zn_tiles[b] = npool.tile(
                        [128, TPB, 128], f32, name=f"zn{b}", tag="zn"
                    )
                zn = zn_tiles[b]
                for t in range(t0, t1):
                    j = TPB * b + t
                    nc.vector.tensor_scalar_mul(
                        zn[:, t, :], zin_tiles[b][:, t, :], inv[:, j : j + 1]
                    )

            def build_half(b, h):
                zn = zn_tiles[b]
                ps = qpool.tile([128, 1024], f32, tag="mm")
                for i in range(8):
                    t = 8 * h + i
                    nc.tensor.transpose(
                        ps[:, 128 * i : 128 * (i + 1)], zn[:, t, :], eye_sb[:]
                    )
                c0 = 2048 * b + 1024 * h
                nc.vector.tensor_copy(znT[:, c0 : c0 + 1024], ps[:])

            # --- prologue: batch 0 only ---
            squares(0, 0, TPB)
            norms(0)
            tsm(0, 0, 8)
            build_half(0, 0)
            tsm(0, 8, TPB)
            build_half(0, 1)

            # --- main loop: q-outer, r-inner; build batch q+1 under chunk q ---
            for q in range(QB):
                b = q + 1
                for r in range(RB):
                    lhsT = znT[:, 128 * r : 128 * (r + 1)]
                    ps = qpool.tile([128, 2048], f32, tag="mm")
                    for k in range(KB):
                        c0 = 2048 * q + 512 * k
                        nc.tensor.matmul(
                            ps[:, 512 * k : 512 * (k + 1)],
                            lhsT,
                            znT[:, c0 : c0 + 512],
                            start=True,
                            stop=True,
                        )
                    if q == 0:
                        sub = ps[:, 128 * r : 128 * (r + 1)]
                        nc.vector.tensor_add(sub, sub, negI_sb[:])
                    if q == 2:
                        scr = spool.tile([128, 128], f32)
                        nc.vector.tensor_mul(
                            scr[:], ps[:, 128 * r : 128 * (r + 1)], eye_sb[:]
                        )
                        nc.vector.reduce_sum(
                            pos[:, r : r + 1], scr[:], axis=AX.X
                        )
                    nc.scalar.activation(
                        ps[:],
                        ps[:],
                        AF.Exp,
                        scale=INV_T,
                        accum_out=sexp[:, r, q : q + 1],
                    )
                    if b < NBATCH:
                        if r == 0:
                            squares(b, 0, 6)
                        elif r == 1:
                            squares(b, 6, 12)
                        elif r == 2:
                            squares(b, 12, TPB)
                        elif r == 3:
                            norms(b)
                            tsm(b, 0, 8)
                        elif r == 4:
                            tsm(b, 8, TPB)
                        elif r == 5:
                            build_half(b, 0)
                        elif r == 6:
                            build_half(b, 1)

            # --- epilogue ---
            s8 = ppool.tile([128, RB], f32)
            nc.vector.reduce_sum(s8[:], sexp[:], axis=AX.X)
            lse = ppool.tile([128, RB], f32)
            nc.scalar.activation(lse[:], s8[:], AF.Ln)
            poss = ppool.tile([128, RB], f32)
            nc.scalar.mul(poss[:], pos[:], INV_T)
            acc = ppool.tile([128, RB], f32)
            nc.vector.tensor_sub(acc[:], lse[:], poss[:])
            tot = ppool.tile([128, 1], f32)
            nc.vector.reduce_sum(tot[:], acc[:], axis=AX.X)
            psf = qpool.tile([128, 2048], f32, tag="mm")
            nc.tensor.matmul(
                psf[0:1, 0:1], ones_sb[:], tot[:], start=True, stop=True
            )
            res = ppool.tile([1, 1], f32)
            nc.scalar.mul(res[:], psf[0:1, 0:1], 1.0 / N)
            nc.gpsimd.dma_start(loss_dram[:], res[:])

    nc.compile()
    return nc


def get_nc():
    if "nc" not in _cache:
        _cache["nc"] = build()
    return _cache["nc"]


def make_in_maps(z_i: np.ndarray, z_j: np.ndarray):
    z = np.concatenate(
        [np.asarray(z_i, np.float32), np.asarray(z_j, np.float32)], axis=0
    )
    return [
        {"z_roll": np.ascontiguousarray(np.roll(z, -RPC * c, axis=0))}
        for c in range(N_CORES)
    ]


def kernel(**inputs) -> np.ndarray:
    in_maps = make_in_maps(inputs["z_i"], inputs["z_j"])
    nc = get_nc()
    res = run_bass_kernel_spmd(nc, in_maps, list(range(N_CORES)))
    kernel.last_results = res
    total = np.float32(0.0)
    for r in res.results:
        total = np.float32(total + np.float32(np.asarray(r["loss_part"]).reshape(())))
    return np.float32(total)


# revision 11
# speedup vs baseline: 1.1823x; 1.0367x over previous
"""NT-Xent contrastive loss on 8 Trainium2 NeuronCores.

Math (reference): z = [z_i; z_j] (N=8192, D=128), zn = z/||z||,
sim = zn@zn.T / 0.1.  Row loss_i = logsumexp_{j!=i} sim[i,j] - sim[i, pos(i)],
loss = mean_i loss_i.

Sharding: rolled-column trick.  Core c receives z rolled by -1024*c rows.
Its 1024 local rows are rolled rows 0..1023; in rolled coordinates the
self column of local row i is i and the positive column is i + 4096 on
EVERY core, so a single static SPMD program works with no collectives.
The self logit is suppressed by adding -5 to the diagonal cosine
(logit -40 -> exp ~4e-18, negligible).  Host sums the 8 partial means.

Per-core schedule (V2, software-pipelined):
  prologue: batch 0 (rows 0..2047) only: DMA (4 sub-DMAs), row sum-sq on
            DVE (tensor_mul+reduce_sum -- tensor_tensor_reduce hangs the
            DVE on HW), 1/norm = exp(-0.5*ln(ssq)) on ACT, scale rows,
            PE-transpose into znT[:, :2048] (f32r producer rule).
  main:     q-outer / r-inner.  Chunk q's 8 row-block exps (ACT-bound,
            ~17us) hide batch q+1's stage-A work, interleaved into the
            engine queues mid-chunk: squares at r=0..2, norms+scale at
            r=3..4, transpose+copy halves at r=5..6.  Diag shift at q==0,
            positive extraction at q==2 (cols 4096..5119).
  epilog:   lse = ln(sum exp), row loss = lse - 10*pos_cos, reduce over
            free dim then partitions (ones-vector matmul), scale by 1/8192.
"""

import os
import sys

import numpy as np

_TRN_REPO = "/opt/trn_rl_repo"
if _TRN_REPO not in sys.path:
    sys.path.insert(0, _TRN_REPO)

from concourse import bacc, bass, mybir, tile
from concourse.bass_utils import run_bass_kernel_spmd

B = 4096
D = 128
N = 2 * B
N_CORES = 8
RPC = N // N_CORES  # 1024 rows per core
INV_T = 10.0
DIAG_SHIFT = -5.0

NBATCH = 4  # stage-A batches of 2048 rows
TPB = 16    # 128-row tiles per batch
RB = 8      # row blocks per core (128 rows each)
QB = 4      # 2048-wide column chunks
KB = 4      # 512-wide matmuls per chunk

_cache: dict = {}


def build():
    f32 = mybir.dt.float32
    f32r = mybir.dt.float32r
    AX = mybir.AxisListType
    OP = mybir.AluOpType
    AF = mybir.ActivationFunctionType

    nc = bacc.Bacc(
        "TRN2", target_bir_lowering=False, debug=False, num_devices=N_CORES
    )

    # All ACT funcs used here (ln/exp/copy/square/identity) coexist in the
    # natural_log_exp_and_others table, but the default per-func table
    # choice splits them across tables, costing a 1.3us ACT_TABLE_LOAD at
    # every ln<->exp transition (7 loads/run).  get_activation_tables is
    # functools.cached and bacc reads the same dict object at compile time,
    # so strip this kernel's funcs from every other table (indices
    # preserved) to pin them all to one table -> a single load.
    tabs = bacc.get_activation_tables(nc.m.arch)
    pinned = set(tabs["natural_log_exp_and_others"])
    for k in tabs:
        if k != "natural_log_exp_and_others":
            tabs[k] = tabs[k] - pinned

    z_dram = nc.dram_tensor("z_roll", [N, D], f32, kind="ExternalInput")
    loss_dram = nc.dram_tensor("loss_part", [1, 1], f32, kind="ExternalOutput")

    eye_np = np.eye(128, dtype=np.float32)
    eye_dram = nc.inline_tensor(eye_np, name="eye128")
    negI_dram = nc.inline_tensor(
        (DIAG_SHIFT * eye_np).astype(np.float32), name="negI128"
    )
    ones_dram = nc.inline_tensor(np.ones((128, 1), np.float32), name="ones128")

    with tile.TileContext(nc) as tc:
        with (
            tc.tile_pool(name="const", bufs=1) as cpool,
            tc.tile_pool(name="zin", bufs=NBATCH) as zpool,
            tc.tile_pool(name="zn", bufs=2) as npool,
            tc.tile_pool(name="persist", bufs=1) as ppool,
            tc.tile_pool(name="scr", bufs=2) as spool,
            tc.tile_pool(name="psum", bufs=2, space=bass.MemorySpace.PSUM) as qpool,
        ):
            eye_sb = cpool.tile([128, 128], f32)
            negI_sb = cpool.tile([128, 128], f32)
            ones_sb = cpool.tile([128, 1], f32)
            nc.gpsimd.dma_start(eye_sb[:], eye_dram[:])
            nc.gpsimd.dma_start(negI_sb[:], negI_dram[:])
            nc.gpsimd.dma_start(ones_sb[:], ones_dram[:])

            ssq = ppool.tile([128, NBATCH * TPB], f32)
            lnssq = ppool.tile([128, NBATCH * TPB], f32)
            inv = ppool.tile([128, NBATCH * TPB], f32)
            # znT is typed f32r: walrus requires the producer of an
            # fp32r-matmul operand to round to fp32r at write time.
            znT = ppool.tile([128, N], f32r)
            sexp = ppool.tile([128, RB, QB], f32)
            pos = ppool.tile([128, RB], f32)

            # all input DMAs up front; batch 0 first, 4 sub-DMAs per batch
            zin_tiles = []
            for b in range(NBATCH):
                zin = zpool.tile([128, TPB, 128], f32)
                zin_tiles.append(zin)
                for s in range(4):
                    r0 = 2048 * b + 512 * s
                    src = z_dram[r0 : r0 + 512, :].rearrange(
                        "(t p) d -> p t d", p=128
                    )
                    nc.gpsimd.dma_start(zin[:, 4 * s : 4 * s + 4, :], src)

            def squares(b, t0, t1):
                for t in range(t0, t1):
                    j = TPB * b + t
                    scr = spool.tile([128, 128], f32)
                    nc.vector.tensor_mul(
                        scr[:], zin_tiles[b][:, t, :], zin_tiles[b][:, t, :]
                    )
                    nc.vector.reduce_sum(ssq[:, j : j + 1], scr[:], axis=AX.X)

            def norms(b):
                # 1/||z|| = exp(-0.5*ln(ssq)); stays in the Ln/Exp ACT table.
                j0 = TPB * b
                nc.scalar.activation(
                    lnssq[:, j0 : j0 + TPB], ssq[:, j0 : j0 + TPB], AF.Ln
                )
                nc.scalar.activation(
                    inv[:, j0 : j0 + TPB], lnssq[:, j0 : j0 + TPB],
                    AF.Exp, scale=-0.5,
                )

            zn_tiles = {}

            def tsm(b, t0, t1):
                if b not in zn_tiles:
                    zn_tiles[b] = npool.tile(
                        [128, TPB, 128], f32, name=f"zn{b}", tag="zn"
                    )
                zn = zn_tiles[b]
                for t in range(t0, t1):
                    j = TPB * b + t
                    nc.vector.tensor_scalar_mul(
                        zn[:, t, :], zin_tiles[b][:, t, :], inv[:, j : j + 1]
                    )

            def build_half(b, h):
                zn = zn_tiles[b]
                ps = qpool.tile([128, 1024], f32, tag="mm")
                for i in range(8):
                    t = 8 * h + i
                    nc.tensor.transpose(
                        ps[:, 128 * i : 128 * (i + 1)], zn[:, t, :], eye_sb[:]
                    )
                c0 = 2048 * b + 1024 * h
                nc.vector.tensor_copy(znT[:, c0 : c0 + 1024], ps[:])

            # --- prologue: batch 0 only ---
            squares(0, 0, TPB)
            norms(0)
            tsm(0, 0, 8)
            build_half(0, 0)
            tsm(0, 8, TPB)
            build_half(0, 1)

            # --- main loop: q-outer, r-inner; build batch q+1 under chunk q ---
            for q in range(QB):
                b = q + 1
                for r in range(RB):
                    lhsT = znT[:, 128 * r : 128 * (r + 1)]
                    ps = qpool.tile([128, 2048], f32, tag="mm")
                    for k in range(KB):
                        c0 = 2048 * q + 512 * k
                        nc.tensor.matmul(
                            ps[:, 512 * k : 512 * (k + 1)],
                            lhsT,
                            znT[:, c0 : c0 + 512],
                            start=True,
                            stop=True,
                        )
                    if q == 0:
                        sub = ps[:, 128 * r : 128 * (r + 1)]
                        nc.vector.tensor_add(sub, sub, negI_sb[:])
                    if q == 2:
                        scr = spool.tile([128, 128], f32)
                        nc.vector.tensor_mul(
                            scr[:], ps[:, 128 * r : 128 * (r + 1)], eye_sb[:]
                        )
                        nc.vector.reduce_sum(
                            pos[:, r : r + 1], scr[:], axis=AX.X
                        )
                    nc.scalar.activation(
                        ps[:],
                        ps[:],
                        AF.Exp,
                        scale=INV_T,
                        accum_out=sexp[:, r, q : q + 1],
                    )
                    if b < NBATCH:
                        if r == 0:
                            squares(b, 0, 6)
                        elif r == 1:
                            squares(b, 6, 12)
                        elif r == 2:
                            squares(b, 12, TPB)
                        elif r == 3:
                            norms(b)
                            tsm(b, 0, 8)
                        elif r == 4:
                            tsm(b, 8, TPB)
                        elif r == 5:
                            build_half(b, 0)
                        elif r == 6:
                            build_half(b, 1)

            # --- epilogue ---
            s8 = ppool.tile([128, RB], f32)
            nc.vector.reduce_sum(s8[:], sexp[:], axis=AX.X)
            lse = ppool.tile([128, RB], f32)
            nc.scalar.activation(lse[:], s8[:], AF.Ln)
            poss = ppool.tile([128, RB], f32)
            nc.scalar.mul(poss[:], pos[:], INV_T)
            acc = ppool.tile([128, RB], f32)
            nc.vector.tensor_sub(acc[:], lse[:], poss[:])
            tot = ppool.tile([128, 1], f32)
            nc.vector.reduce_sum(tot[:], acc[:], axis=AX.X)
            psf = qpool.tile([128, 2048], f32, tag="mm")
            nc.tensor.matmul(
                psf[0:1, 0:1], ones_sb[:], tot[:], start=True, stop=True
            )
            res = ppool.tile([1, 1], f32)
            nc.scalar.mul(res[:], psf[0:1, 0:1], 1.0 / N)
            nc.gpsimd.dma_start(loss_dram[:], res[:])

    nc.compile()
    return nc


def get_nc():
    if "nc" not in _cache:
        _cache["nc"] = build()
    return _cache["nc"]


def make_in_maps(z_i: np.ndarray, z_j: np.ndarray):
    z = np.concatenate(
        [np.asarray(z_i, np.float32), np.asarray(z_j, np.float32)], axis=0
    )
    return [
        {"z_roll": np.ascontiguousarray(np.roll(z, -RPC * c, axis=0))}
        for c in range(N_CORES)
    ]


def kernel(**inputs) -> np.ndarray:
    in_maps = make_in_maps(inputs["z_i"], inputs["z_j"])
    nc = get_nc()
    res = run_bass_kernel_spmd(nc, in_maps, list(range(N_CORES)))
    kernel.last_results = res
    total = np.float32(0.0)
    for r in res.results:
        total = np.float32(total + np.float32(np.asarray(r["loss_part"]).reshape(())))
    return np.float32(total)
